# revision 1
# baseline (speedup 1.0000x reference)
"""Multi-head causal attention (B=2, T=2048, C=1024, H=16) on 8 Trainium2
NeuronCores, tensor-parallel over heads (2 heads per core).

Layout strategy (everything column-major on device, i.e. feature = SBUF
partition dim, token = free dim):
  - host feeds xT [C, B*T] in bf16; per-core w_qkv column slices / w_out row
    slice (bf16).
  - phase 1: QT/KT/VT [128, 4096] = w_c.T @ xT   (accumulate over 8 k-tiles)
  - phase 1.5: PE-transpose V into natural [token, dim] layout, interleaved
    with a ones column per head (row-sums of attention weights come free
    in the AV matmul).
  - phase 2/3 (flash-style, causal tiles skipped): per (batch, q-chunk 512):
      ST[k,q] = KT_tile.T @ QT_chunk  -> +mask on diagonal tiles (DVE)
      PT = exp(ST/8) (ScalarE, PSUM->SBUF, bf16)
      OT_aug[65, q] += Vaug_tile.T @ PT  (row 64 = softmax denominator),
        AV delayed 2 k-tiles behind scores to hide exp latency.
    normalize: sums -> outer-product broadcast (fp32r) -> fast reciprocal
    (DVE, fp32) -> multiply; then
      yT[m-tile, q-chunk] = sum_h wout_h.T @ OT_h  -> direct PSUM->HBM DMA.
  - host: sum 8 partial yT, transpose, add b_out.

Matmuls run in bf16 (fp32 PSUM accumulate); the softmax normalization
chain stays fp32/f32r so per-element output scaling is accurate.
"""

import os
import sys

for _p in ("/opt/trn_rl_repo", "/root/.axon_site/_ro/trn_rl_repo"):
    if os.path.isdir(_p) and _p not in sys.path:
        sys.path.insert(0, _p)

import ml_dtypes
import numpy as np

import concourse.bacc as bacc
import concourse.bass as bass
import concourse.mybir as mybir
import concourse.tile as tile
from concourse.bass_utils import run_bass_kernel_spmd
from concourse.masks import make_identity

B, T, C, H, D = 2, 2048, 1024, 16, 64
NCORES = 8
BT = B * T                      # 4096 flattened tokens
TC = 512                        # token chunk (matmul free dim)
NTC = BT // TC                  # 8 token chunks
FP = mybir.dt.float32
FPR = mybir.dt.float32r
BF = mybir.dt.bfloat16
ACT = mybir.ActivationFunctionType
NEG = -1.0e9
AV_DELAY = 2                    # k-tiles the AV matmul trails the scores

LAST_RESULTS = None             # stashed BassKernelResults for test harness


def build_nc():
    nc = bacc.Bacc(None, target_bir_lowering=False, debug=False)

    xt = nc.declare_dram_parameter("xt", [C, BT], BF, isOutput=False)
    wc = nc.declare_dram_parameter("wc", [C, 384], BF, isOutput=False)
    wout = nc.declare_dram_parameter("wout", [128, C], BF, isOutput=False)
    bqkv = nc.declare_dram_parameter("bqkv", [128, 3], FP, isOutput=False)
    masks = nc.declare_dram_parameter("masks", [512, 512], FP, isOutput=False)
    ones = nc.declare_dram_parameter("ones", [128, 64], BF, isOutput=False)
    onesr = nc.declare_dram_parameter("onesr", [1, 64], FP, isOutput=False)
    yt = nc.declare_dram_parameter("yt", [C, BT], FP, isOutput=True)

    with tile.TileContext(nc) as tc:
        with (
            tc.tile_pool(name="const", bufs=1) as cpool,
            tc.tile_pool(name="big", bufs=1) as bigpool,
            tc.tile_pool(name="sb", bufs=2) as sbpool,
            tc.tile_pool(name="ps", bufs=2, space="PSUM") as pspool,
        ):
            # ---- constants ----
            wc_sb = cpool.tile([128, 8 * 384], BF)      # [cin, k*384 + g*128 + col]
            nc.sync.dma_start(
                out=wc_sb[:].rearrange("b (a c) -> b a c", a=8),
                in_=wc.rearrange("(a b) c -> b a c", a=8),
            )
            # w_out split per local head so the out-projection can contract
            # each head from partition base 0
            wout_sbs = (cpool.tile([64, C], BF, name="wout0"),
                        cpool.tile([64, C], BF, name="wout1"))
            nc.sync.dma_start(out=wout_sbs[0][:], in_=wout[0:64, :])
            nc.sync.dma_start(out=wout_sbs[1][:], in_=wout[64:128, :])
            bq_sb = cpool.tile([128, 3], FP)
            nc.sync.dma_start(out=bq_sb[:], in_=bqkv[:, :])
            masks_sb = cpool.tile([128, 4 * 512], FP)
            nc.sync.dma_start(
                out=masks_sb[:].rearrange("b (a c) -> b a c", a=4),
                in_=masks.rearrange("(a b) c -> b a c", a=4),
            )
            onesr_sb = cpool.tile([1, 64], FPR)
            nc.sync.dma_start(out=onesr_sb[:], in_=onesr.bitcast(FPR)[:, :])
            ident = cpool.tile([128, 128], FP)
            make_identity(nc, ident)

            # ---- persistent intermediates ----
            QT = bigpool.tile([128, BT], BF)
            KT = bigpool.tile([128, BT], BF)
            VT = bigpool.tile([128, BT], FP)
            # V in [token, dim] layout, 130 cols per 128-token block:
            # [V_h0 (64) | ones | V_h1 (64) | ones]
            vaug = bigpool.tile([128, 32 * 130], BF)
            nc.sync.dma_start(
                out=vaug[:].rearrange("p (j a c) -> p j a c", a=2, c=65)[
                    :, :, :, 64:65],
                in_=ones.rearrange("p (j a c) -> p j a c", a=2, c=1)[:, 0:32],
            )

            qkvT = (QT, KT, VT)

            for tcx in range(NTC):
                b, qc = divmod(tcx, 4)
                t0 = tcx * TC

                # ---- phase 1: QKV projection for this token chunk ----
                xts = []
                for k in range(8):
                    xtile = sbpool.tile([128, TC], BF, tag="xt", bufs=10)
                    nc.sync.dma_start(
                        out=xtile[:],
                        in_=xt[k * 128:(k + 1) * 128, t0:t0 + TC],
                    )
                    xts.append(xtile)
                for g in range(3):
                    ps = pspool.tile([128, TC], FP, tag="q", bufs=2)
                    for k in range(8):
                        nc.tensor.matmul(
                            ps[:],
                            wc_sb[:, k * 384 + g * 128:k * 384 + (g + 1) * 128],
                            xts[k][:],
                            start=(k == 0),
                            stop=(k == 7),
                        )
                    nc.scalar.activation(
                        qkvT[g][:, t0:t0 + TC], ps[:], ACT.Identity,
                        bias=bq_sb[:, g:g + 1],
                    )

                # ---- phase 1.5: transpose this chunk's V into vaug ----
                for j in range(4):
                    jj = tcx * 4 + j
                    tp = pspool.tile([128, 128], FP, tag="q", bufs=2, name="tp")
                    nc.tensor.transpose(
                        tp[:], VT[:, jj * 128:(jj + 1) * 128], ident[:]
                    )
                    nc.vector.tensor_copy(
                        vaug[:].rearrange("p (j a c) -> p j a c", a=2, c=65)[
                            :, jj, :, 0:64],
                        tp[:].rearrange("p (a c) -> p a c", c=64),
                    )

                # ---- phase 2/3: causal attention for (b, qc) ----
                n_kt = 4 * (qc + 1)
                otps = [
                    pspool.tile([65, TC], FP, tag="av", bufs=2, name=f"otp{_h}")
                    for _h in range(2)
                ]
                pts = {}

                def emit_av(j, kg0):
                    for h in range(2):
                        nc.tensor.matmul(
                            otps[h][:],
                            vaug[:, kg0 * 130 + h * 65:kg0 * 130 + h * 65 + 65],
                            pts.pop((j, h))[:],
                            start=(j == 0), stop=(j == n_kt - 1),
                            skip_group_check=True,
                        )

                for kt in range(n_kt):
                    kg = b * 16 + kt
                    for h in range(2):
                        sp = pspool.tile([128, TC], FP, tag="s", bufs=2)
                        nc.tensor.matmul(
                            sp[:],
                            KT[h * 64:(h + 1) * 64, kg * 128:(kg + 1) * 128],
                            QT[h * 64:(h + 1) * 64, t0:t0 + TC],
                            start=True, stop=True,
                        )
                        if kt >= 4 * qc:
                            v = kt - 4 * qc
                            nc.vector.tensor_add(
                                sp[:], sp[:], masks_sb[:, v * 512:(v + 1) * 512]
                            )
                        pt = sbpool.tile([128, TC], BF, tag="pt",
                                         bufs=2 * (AV_DELAY + 1))
                        nc.scalar.activation(pt[:], sp[:], ACT.Exp, scale=0.125)
                        pts[(kt, h)] = pt
                    if kt >= AV_DELAY:
                        emit_av(kt - AV_DELAY, b * 16 + kt - AV_DELAY)
                for j in range(max(n_kt - AV_DELAY, 0), n_kt):
                    emit_av(j, b * 16 + j)

                # ---- normalize: 1/rowsum broadcast, per head ----
                ots = []
                for h in range(2):
                    rcsum = sbpool.tile([1, TC], FPR, tag=f"rc{h}", bufs=2,
                                        name=f"rc{h}")
                    with nc.allow_low_precision(reason="softmax sums f32r"):
                        nc.scalar.copy(rcsum[:], otps[h][64:65, :])
                    bch = pspool.tile([64, TC], FP, tag="s", bufs=2,
                                      name=f"bc{h}")
                    nc.tensor.matmul(bch[:], onesr_sb[0:1, :], rcsum[:],
                                     start=True, stop=True)
                    bcs = sbpool.tile([64, TC], FP, tag=f"bcs{h}", bufs=2,
                                      name=f"bcs{h}")
                    nc.vector.reciprocal_approx_fast(out=bcs[:], in_=bch[:])
                    oth = sbpool.tile([64, TC], BF, tag=f"ot{h}", bufs=2,
                                      name=f"ot{h}")
                    nc.vector.tensor_mul(oth[:], otps[h][0:64, :], bcs[:])
                    ots.append(oth)

                # ---- phase 4: output projection (contract heads) ----
                for m in range(8):
                    yp = pspool.tile([128, TC], FP, tag="y", bufs=2)
                    for h in range(2):
                        nc.tensor.matmul(
                            yp[:],
                            wout_sbs[h][:, m * 128:(m + 1) * 128],
                            ots[h][:],
                            start=(h == 0), stop=(h == 1),
                        )
                    ys = sbpool.tile([128, TC], FP, tag="ys", bufs=4)
                    if m % 2 == 0:
                        nc.scalar.copy(ys[:], yp[:])
                    else:
                        nc.vector.tensor_copy(ys[:], yp[:])
                    nc.sync.dma_start(
                        out=yt[m * 128:(m + 1) * 128, t0:t0 + TC], in_=ys[:]
                    )
    nc.compile()
    return nc


def make_in_maps(x, w_qkv, b_qkv):
    x = np.ascontiguousarray(np.asarray(x, np.float32).reshape(BT, C))
    xT = np.ascontiguousarray(x.T).astype(ml_dtypes.bfloat16)
    w_qkv = np.asarray(w_qkv, np.float32)
    b_qkv = np.asarray(b_qkv, np.float32)

    mask = np.empty((512, 512), np.float32)
    for v in range(4):
        kk = np.arange(128)[:, None] + 128 * v
        qq = np.arange(512)[None, :]
        mask[v * 128:(v + 1) * 128] = np.where(kk <= qq, 0.0, NEG)

    in_maps = []
    for c in range(NCORES):
        sl = slice(c * 128, (c + 1) * 128)
        wcs = np.concatenate(
            [w_qkv[:, sl], w_qkv[:, 1024:][:, sl], w_qkv[:, 2048:][:, sl]], axis=1
        )
        bq = np.stack(
            [b_qkv[sl], b_qkv[1024:][sl], b_qkv[2048:][sl]], axis=1
        )
        in_maps.append({
            "xt": xT,
            "wc": np.ascontiguousarray(wcs).astype(ml_dtypes.bfloat16),
            "wout": None,  # filled by caller (needs w_out)
            "bqkv": np.ascontiguousarray(bq),
            "masks": mask,
            "ones": np.ones((128, 64), ml_dtypes.bfloat16),
            "onesr": np.ones((1, 64), np.float32),
        })
    return in_maps


_NC_CACHE = None


def kernel(x, w_qkv, b_qkv, w_out, b_out):
    global _NC_CACHE, LAST_RESULTS
    if _NC_CACHE is None:
        _NC_CACHE = build_nc()
    nc = _NC_CACHE

    w_out = np.asarray(w_out, np.float32)
    in_maps = make_in_maps(x, w_qkv, b_qkv)
    for c in range(NCORES):
        in_maps[c]["wout"] = np.ascontiguousarray(
            w_out[c * 128:(c + 1) * 128, :]).astype(ml_dtypes.bfloat16)

    res = run_bass_kernel_spmd(
        nc, in_maps, list(range(NCORES)),
        trace=bool(os.environ.get("BASS_TRACE")),
    )
    LAST_RESULTS = res

    acc = np.zeros((C, BT), np.float64)
    for out_map in res.results:
        acc += out_map["yt"].astype(np.float64)
    y = acc.T.astype(np.float32) + np.asarray(b_out, np.float32)[None, :]
    return y.reshape(B, T, C)



# revision 9
# speedup vs baseline: 1.1416x; 1.1416x over previous
"""Multi-head causal attention (B=2, T=2048, C=1024, H=16) on 8 Trainium2
NeuronCores, tensor-parallel over heads (2 heads per core).

v1 rewrite of the working baseline, targeting sustained Tensor-engine
occupancy (HAM stays un-throttled) and reduced ScalarE/DVE overhead:

  - software-pipelined emission: phase-1 (QKV proj) of chunk i+1 and
    phase-4 (out proj) of chunk i-1 are interleaved as "filler" PE work
    inside chunk i's attention k-loop; the last AV_DELAY AV matmuls and
    the softmax normalization are deferred past the chunk boundary so
    the PE never waits on the exp pipeline tail.
  - exp batched 1024-wide: both heads' score tiles land in one
    [128, 1024] PSUM tile (2 banks) and one ACTIVATE handles them.
  - out-projection contracts 128 (both heads stacked) instead of 2x64.
  - phase-1 bias-adds run on DVE (tensor_scalar add); ScalarE does
    (almost) nothing but exp.
  - x kept fully SBUF-resident (8 x [128, 4096] bf16), output written
    bf16, mask tiles bf16, V path bf16 end-to-end.

Host-side layouts are pre-tiled for contiguous DMA rows; the 8 partial
[C, BT] outputs are summed (and bias added) on the host.
"""

import os
import sys
from collections import deque

for _p in ("/opt/trn_rl_repo", "/root/.axon_site/_ro/trn_rl_repo"):
    if os.path.isdir(_p) and _p not in sys.path:
        sys.path.insert(0, _p)

import ml_dtypes
import numpy as np

import concourse.bacc as bacc
import concourse.bass as bass
import concourse.mybir as mybir
import concourse.tile as tile
from concourse.bass_utils import run_bass_kernel_spmd
from concourse.masks import make_identity

B, T, C, H, D = 2, 2048, 1024, 16, 64
NCORES = 8
BT = B * T                      # 4096 flattened tokens
TC = 512                        # token chunk (matmul free dim)
NTC = BT // TC                  # 8 token chunks
FP = mybir.dt.float32
FPR = mybir.dt.float32r
BF = mybir.dt.bfloat16
ACT = mybir.ActivationFunctionType
NEG = -1.0e9
AV_DELAY = 2                    # k-tiles the AV matmul trails the scores

LAST_RESULTS = None             # stashed BassKernelResults for test harness


def build_nc():
    nc = bacc.Bacc(None, target_bir_lowering=False, debug=False)

    xt = nc.declare_dram_parameter("xt", [C, BT], BF, isOutput=False)
    wc = nc.declare_dram_parameter("wc", [128, 3072], BF, isOutput=False)
    wout = nc.declare_dram_parameter("wout", [128, C], BF, isOutput=False)
    bqkv = nc.declare_dram_parameter("bqkv", [128, 3], FP, isOutput=False)
    masks = nc.declare_dram_parameter("masks", [128, 4096], BF, isOutput=False)
    onesr = nc.declare_dram_parameter("onesr", [1, 64], FP, isOutput=False)
    # yt[i*128 + p, m*512 + t] = y_partial[m*128 + p, i*512 + t]
    yt = nc.declare_dram_parameter("yt", [C, BT], BF, isOutput=True)

    with tile.TileContext(nc) as tc:
        with (
            tc.tile_pool(name="const", bufs=1) as cpool,
            tc.tile_pool(name="big", bufs=1) as bigpool,
            tc.tile_pool(name="sb", bufs=2) as sbpool,
            tc.tile_pool(name="ps", bufs=2, space="PSUM") as pspool,
        ):
            # ---- constants; DMA order = startup order ----
            wc_sb = cpool.tile([128, 3072], BF)
            nc.sync.dma_start(out=wc_sb[:], in_=wc[:, :])
            bq_sb = cpool.tile([128, 3], FP)
            nc.sync.dma_start(out=bq_sb[:], in_=bqkv[:, :])
            # x resident: chunk-0 column span first so compute starts early
            xks = []
            for k in range(8):
                xk = cpool.tile([128, BT], BF, name=f"xk{k}")
                nc.sync.dma_start(out=xk[:, 0:TC],
                                  in_=xt[k * 128:(k + 1) * 128, 0:TC])
                xks.append(xk)
            for k in range(8):
                nc.sync.dma_start(out=xks[k][:, TC:BT],
                                  in_=xt[k * 128:(k + 1) * 128, TC:BT])
            wout_sb = cpool.tile([128, C], BF)
            nc.sync.dma_start(out=wout_sb[:], in_=wout[:, :])
            masks_sb = cpool.tile([128, 4096], BF)
            nc.sync.dma_start(out=masks_sb[:], in_=masks[:, :])
            onesr_sb = cpool.tile([1, 64], FPR)
            nc.sync.dma_start(out=onesr_sb[:], in_=onesr.bitcast(FPR)[:, :])
            ident = cpool.tile([128, 128], BF)
            make_identity(nc, ident)

            # ---- persistent intermediates ----
            QT = bigpool.tile([128, BT], BF)
            KT = bigpool.tile([128, BT], BF)
            # V in [token, dim] layout, 130 cols per 128-token block:
            # [V_h0 (64) | ones | V_h1 (64) | ones]; memset once puts the
            # ones columns in place, transposed V overwrites the rest.
            vaug = bigpool.tile([128, 32 * 130], BF)
            nc.vector.memset(vaug[:], 1.0)

            vts = {}

            def make_phase1_units(i):
                """QKV projection + V transpose for chunk i, as PE-dense
                closures suitable for filler interleaving."""
                t0 = i * TC

                def g_unit(g):
                    def emit():
                        ps = pspool.tile([128, TC], FP, tag="m", bufs=2,
                                         name=f"p1q{i}g{g}")
                        for k in range(8):
                            nc.tensor.matmul(
                                ps[:],
                                wc_sb[:, k * 384 + g * 128:
                                      k * 384 + (g + 1) * 128],
                                xks[k][:, t0:t0 + TC],
                                start=(k == 0), stop=(k == 7),
                            )
                        if g < 2:
                            dest = (QT, KT)[g][:, t0:t0 + TC]
                        else:
                            vt = sbpool.tile([128, TC], BF, tag="vt", bufs=2,
                                             name=f"vt{i}")
                            vts[i] = vt
                            dest = vt[:]
                        nc.vector.tensor_scalar_add(
                            out=dest, in0=ps[:], scalar1=bq_sb[:, g:g + 1])
                    return emit

                def t_unit():
                    vt = vts.pop(i)
                    for j in range(4):
                        jj = i * 4 + j
                        tp = pspool.tile([128, 128], BF, tag="m", bufs=2,
                                         name=f"tp{i}_{j}")
                        nc.tensor.transpose(
                            tp[:], vt[:, j * 128:(j + 1) * 128], ident[:])
                        nc.vector.tensor_copy(
                            vaug[:].rearrange(
                                "p (j a c) -> p j a c", a=2, c=65)[
                                :, jj, :, 0:64],
                            tp[:].rearrange("p (a c) -> p a c", c=64),
                        )

                return [g_unit(0), g_unit(1), g_unit(2), t_unit]

            filler = deque()

            def pop_fillers(k):
                for _ in range(k):
                    if filler:
                        filler.popleft()()

            # state deferred from the previous chunk
            pending = None  # (i, b, n, otps, pts)

            def finish_pending():
                """Tail AVs + normalization for the previous chunk; returns
                phase-4 filler units for it."""
                nonlocal pending
                if pending is None:
                    return []
                pi, pb, pn, potps, ppts = pending
                pending = None
                for j in range(max(pn - AV_DELAY, 0), pn):
                    kgj = pb * 16 + j
                    for h in range(2):
                        nc.tensor.matmul(
                            potps[h][:],
                            vaug[:, kgj * 130 + h * 65:kgj * 130 + h * 65 + 65],
                            ppts[j][:, h * TC:(h + 1) * TC],
                            start=(j == 0), stop=(j == pn - 1),
                            skip_group_check=True,
                        )
                # normalization: 1/rowsum, heads stacked into [128, TC]
                rcs = []
                for h in range(2):
                    rch = sbpool.tile([1, TC], FPR, tag=f"rc{h}", bufs=2,
                                      name=f"rc{h}_{pi}")
                    with nc.allow_low_precision(reason="softmax sums f32r"):
                        nc.scalar.copy(rch[:], potps[h][64:65, :])
                    rcs.append(rch)
                ot = sbpool.tile([128, TC], BF, tag="ot", bufs=2,
                                 name=f"ot{pi}")
                for h in range(2):
                    bchh = pspool.tile([64, TC], FP, tag="m", bufs=2,
                                       name=f"bch{h}_{pi}")
                    nc.tensor.matmul(bchh[:], onesr_sb[0:1, :], rcs[h][:],
                                     start=True, stop=True)
                    bcsh = sbpool.tile([64, TC], FP, tag=f"bcs{h}", bufs=2,
                                       name=f"bcs{h}_{pi}")
                    nc.vector.reciprocal_approx_fast(out=bcsh[:], in_=bchh[:])
                    nc.vector.tensor_mul(ot[h * 64:(h + 1) * 64, :],
                                         potps[h][0:64, :], bcsh[:])

                ys = sbpool.tile([128, BT], BF, tag="ys", bufs=2,
                                 name=f"ys{pi}")

                def m_unit(m0):
                    def emit():
                        for m in (m0, m0 + 1):
                            yp = pspool.tile([128, TC], FP, tag="m", bufs=2,
                                             name=f"yp{pi}_{m}")
                            nc.tensor.matmul(
                                yp[:], wout_sb[:, m * 128:(m + 1) * 128],
                                ot[:], start=True, stop=True)
                            if m % 2 == 0:
                                nc.scalar.copy(
                                    ys[:, m * TC:(m + 1) * TC], yp[:])
                            else:
                                nc.vector.tensor_copy(
                                    ys[:, m * TC:(m + 1) * TC], yp[:])
                        if m0 == 6:
                            nc.sync.dma_start(
                                out=yt[pi * 128:(pi + 1) * 128, :], in_=ys[:])
                    return emit

                return [m_unit(0), m_unit(2), m_unit(4), m_unit(6)]

            for tcx in range(NTC):
                b, qc = divmod(tcx, 4)
                t0 = tcx * TC
                n = 4 * (qc + 1)

                if tcx == 0:
                    for u in make_phase1_units(0):
                        u()
                    filler.extend(make_phase1_units(1))
                else:
                    # drain leftovers so phase-1(tcx) is fully emitted
                    pop_fillers(len(filler))
                    filler.extend(finish_pending())
                    if tcx + 1 < NTC:
                        filler.extend(make_phase1_units(tcx + 1))

                otps = (
                    pspool.tile([65, TC], FP, tag="av0", bufs=1,
                                name=f"otp0_{tcx}"),
                    pspool.tile([65, TC], FP, tag="av1", bufs=1,
                                name=f"otp1_{tcx}"),
                )
                pts = {}
                for kt in range(n):
                    kg = b * 16 + kt
                    s = pspool.tile([128, 2 * TC], FP, tag="s", bufs=2,
                                    name=f"s{tcx}_{kt}")
                    for h in range(2):
                        nc.tensor.matmul(
                            s[:, h * TC:(h + 1) * TC],
                            KT[h * 64:(h + 1) * 64, kg * 128:(kg + 1) * 128],
                            QT[h * 64:(h + 1) * 64, t0:t0 + TC],
                            start=True, stop=True,
                        )
                    # adaptive filler drain across remaining k-slots
                    slots = n - kt
                    pop_fillers(-(-len(filler) // slots) if filler else 0)
                    if kt >= n - 4:
                        v = kt - (n - 4)
                        nc.vector.tensor_add(
                            s[:], s[:],
                            masks_sb[:, v * 1024:(v + 1) * 1024])
                    pt = sbpool.tile([128, 2 * TC], BF, tag="pt", bufs=6,
                                     name=f"pt{tcx}_{kt}")
                    nc.scalar.activation(pt[:], s[:], ACT.Exp, scale=0.125)
                    pts[kt] = pt
                    if kt >= AV_DELAY:
                        j = kt - AV_DELAY
                        kgj = b * 16 + j
                        for h in range(2):
                            nc.tensor.matmul(
                                otps[h][:],
                                vaug[:, kgj * 130 + h * 65:
                                     kgj * 130 + h * 65 + 65],
                                pts[j][:, h * TC:(h + 1) * TC],
                                start=(j == 0), stop=(j == n - 1),
                                skip_group_check=True,
                            )
                pending = (tcx, b, n, otps, pts)

            pop_fillers(len(filler))
            for u in finish_pending():
                u()
    nc.compile()
    return nc


def make_in_maps(x, w_qkv, b_qkv, w_out):
    x = np.ascontiguousarray(np.asarray(x, np.float32).reshape(BT, C))
    xT = np.ascontiguousarray(x.T).astype(ml_dtypes.bfloat16)
    w_qkv = np.asarray(w_qkv, np.float32)
    b_qkv = np.asarray(b_qkv, np.float32)
    w_out = np.asarray(w_out, np.float32)

    # masks[p, v*1024 + h*512 + q] = 0 if (v*128 + p) <= q else NEG
    kk = np.arange(128)[:, None, None, None] + 128 * np.arange(4)[None, :, None, None]
    qq = np.arange(512)[None, None, None, :]
    mask = np.where(kk <= qq, 0.0, NEG) * np.ones((1, 1, 2, 1))
    mask = np.ascontiguousarray(
        mask.transpose(0, 1, 2, 3).reshape(128, 4096)).astype(ml_dtypes.bfloat16)

    # wc[p, k*384 + g*128 + j] = w_qkv[k*128 + p, g*1024 + c0 + j]
    w4 = w_qkv.reshape(8, 128, 3, 1024)

    in_maps = []
    for c in range(NCORES):
        c0 = c * 128
        wcs = np.ascontiguousarray(
            w4[:, :, :, c0:c0 + 128].transpose(1, 0, 2, 3).reshape(128, 3072)
        ).astype(ml_dtypes.bfloat16)
        bq = np.ascontiguousarray(
            b_qkv.reshape(3, 1024)[:, c0:c0 + 128].T)
        in_maps.append({
            "xt": xT,
            "wc": wcs,
            "wout": np.ascontiguousarray(
                w_out[c0:c0 + 128, :]).astype(ml_dtypes.bfloat16),
            "bqkv": bq,
            "masks": mask,
            "onesr": np.ones((1, 64), np.float32),
        })
    return in_maps


_NC_CACHE = None


def kernel(x, w_qkv, b_qkv, w_out, b_out):
    global _NC_CACHE, LAST_RESULTS
    if _NC_CACHE is None:
        _NC_CACHE = build_nc()
    nc = _NC_CACHE

    in_maps = make_in_maps(x, w_qkv, b_qkv, w_out)

    res = run_bass_kernel_spmd(
        nc, in_maps, list(range(NCORES)),
        trace=bool(os.environ.get("BASS_TRACE")),
    )
    LAST_RESULTS = res

    acc = np.zeros((C, BT), np.float64)
    for out_map in res.results:
        # yt[i*128 + p, m*512 + t] -> y_partial[m*128 + p, i*512 + t]
        yp = out_map["yt"].astype(np.float32)
        yp = yp.reshape(8, 128, 8, 512).transpose(2, 1, 0, 3).reshape(C, BT)
        acc += yp
    y = acc.T.astype(np.float32) + np.asarray(b_out, np.float32)[None, :]
    return y.reshape(B, T, C)


# revision 20
# speedup vs baseline: 1.2938x; 1.1333x over previous
"""Multi-head causal attention (B=2, T=2048, C=1024, H=16) on 8 Trainium2
NeuronCores, tensor-parallel over heads (2 heads per core).

v1 rewrite of the working baseline, targeting sustained Tensor-engine
occupancy (HAM stays un-throttled) and reduced ScalarE/DVE overhead:

  - software-pipelined emission: phase-1 (QKV proj) of chunk i+1 and
    phase-4 (out proj) of chunk i-1 are interleaved as "filler" PE work
    inside chunk i's attention k-loop; the last AV_DELAY AV matmuls and
    the softmax normalization are deferred past the chunk boundary so
    the PE never waits on the exp pipeline tail.
  - exp batched 1024-wide: both heads' score tiles land in one
    [128, 1024] PSUM tile (2 banks) and one ACTIVATE handles them.
  - out-projection contracts 128 (both heads stacked) instead of 2x64.
  - phase-1 bias-adds run on DVE (tensor_scalar add); ScalarE does
    (almost) nothing but exp.
  - x kept fully SBUF-resident (8 x [128, 4096] bf16), output written
    bf16, mask tiles bf16, V path bf16 end-to-end.

Host-side layouts are pre-tiled for contiguous DMA rows; the 8 partial
[C, BT] outputs are summed (and bias added) on the host.
"""

import os
import sys
from collections import deque

for _p in ("/opt/trn_rl_repo", "/root/.axon_site/_ro/trn_rl_repo"):
    if os.path.isdir(_p) and _p not in sys.path:
        sys.path.insert(0, _p)

import ml_dtypes
import numpy as np

import concourse.bacc as bacc
import concourse.bass as bass
import concourse.mybir as mybir
import concourse.tile as tile
from concourse.bass_utils import run_bass_kernel_spmd
from concourse.masks import make_identity

B, T, C, H, D = 2, 2048, 1024, 16, 64
NCORES = 8
BT = B * T                      # 4096 flattened tokens
TC = 512                        # token chunk (matmul free dim)
NTC = BT // TC                  # 8 token chunks
FP = mybir.dt.float32
FPR = mybir.dt.float32r
BF = mybir.dt.bfloat16
ACT = mybir.ActivationFunctionType
NEG = -1.0e9
AV_DELAY = 2                    # k-tiles the AV matmul trails the scores

LAST_RESULTS = None             # stashed BassKernelResults for test harness


def build_nc():
    nc = bacc.Bacc(None, target_bir_lowering=False, debug=False)

    xt = nc.declare_dram_parameter("xt", [C, BT], BF, isOutput=False)
    wc = nc.declare_dram_parameter("wc", [128, 3072], BF, isOutput=False)
    wout = nc.declare_dram_parameter("wout", [128, C], BF, isOutput=False)
    bqkv = nc.declare_dram_parameter("bqkv", [128, 3], FP, isOutput=False)
    masks = nc.declare_dram_parameter("masks", [128, 4096], BF, isOutput=False)
    onesr = nc.declare_dram_parameter("onesr", [1, 64], FP, isOutput=False)
    # yt[i*128 + p, m*512 + t] = y_partial[m*128 + p, i*512 + t]
    yt = nc.declare_dram_parameter("yt", [C, BT], BF, isOutput=True)

    with tile.TileContext(nc) as tc:
        with (
            tc.tile_pool(name="const", bufs=1) as cpool,
            tc.tile_pool(name="big", bufs=1) as bigpool,
            tc.tile_pool(name="sb", bufs=2) as sbpool,
            tc.tile_pool(name="ps", bufs=2, space="PSUM") as pspool,
        ):
            # ---- constants; DMA order = startup order ----
            # wc split per k-slice so the first phase-1 matmul only waits
            # on a 96 KB transfer; chunk-0 x columns land first.
            wc_sb = cpool.tile([128, 3072], BF)
            xks = []
            for k in range(8):
                xk = cpool.tile([128, BT], BF, name=f"xk{k}")
                xks.append(xk)
            for k in range(8):
                nc.sync.dma_start(out=wc_sb[:, k * 384:(k + 1) * 384],
                                  in_=wc[:, k * 384:(k + 1) * 384])
                nc.sync.dma_start(out=xks[k][:, 0:TC],
                                  in_=xt[k * 128:(k + 1) * 128, 0:TC])
            bq_sb = cpool.tile([128, 3], FP)
            nc.sync.dma_start(out=bq_sb[:], in_=bqkv[:, :])
            for k in range(8):
                nc.sync.dma_start(out=xks[k][:, TC:BT],
                                  in_=xt[k * 128:(k + 1) * 128, TC:BT])
            wout_sb = cpool.tile([128, C], BF)
            nc.sync.dma_start(out=wout_sb[:], in_=wout[:, :])
            masks_sb = cpool.tile([128, 4096], BF)
            nc.sync.dma_start(out=masks_sb[:], in_=masks[:, :])
            onesr_sb = cpool.tile([1, 64], FPR)
            nc.sync.dma_start(out=onesr_sb[:], in_=onesr.bitcast(FPR)[:, :])
            ident = cpool.tile([128, 128], BF)
            make_identity(nc, ident)

            # ---- persistent intermediates ----
            QT = bigpool.tile([128, BT], BF)
            KT = bigpool.tile([128, BT], BF)
            # V in [token, dim] layout, 130 cols per 128-token block:
            # [V_h0 (64) | ones | V_h1 (64) | ones]; memset once puts the
            # ones columns in place, transposed V overwrites the rest.
            vaug = bigpool.tile([128, 32 * 130], BF)
            nc.vector.memset(vaug[:], 1.0)

            vts = {}
            p1state = {}

            def make_phase1_units(i):
                """QKV projection + V transpose for chunk i, split into
                ~2-matmul granules for fine filler interleaving."""
                t0 = i * TC

                def g_granule(g, k0):
                    def emit():
                        if k0 == 0:
                            p1state[(i, g)] = pspool.tile(
                                [128, TC], FP, tag="m", bufs=2,
                                name=f"p1q{i}g{g}")
                        ps = p1state[(i, g)]
                        for k in (k0, k0 + 1):
                            nc.tensor.matmul(
                                ps[:],
                                wc_sb[:, k * 384 + g * 128:
                                      k * 384 + (g + 1) * 128],
                                xks[k][:, t0:t0 + TC],
                                start=(k == 0), stop=(k == 7),
                            )
                        if k0 == 6:
                            del p1state[(i, g)]
                            if g < 2:
                                dest = (QT, KT)[g][:, t0:t0 + TC]
                            else:
                                vt = sbpool.tile([128, TC], BF, tag="vt",
                                                 bufs=2, name=f"vt{i}")
                                vts[i] = vt
                                dest = vt[:]
                            nc.vector.tensor_scalar_add(
                                out=dest, in0=ps[:],
                                scalar1=bq_sb[:, g:g + 1])
                    return emit

                def t_granule(j0):
                    def emit():
                        vt = vts[i]
                        for j in (j0, j0 + 1):
                            jj = i * 4 + j
                            tp = pspool.tile([128, 128], BF, tag="m", bufs=2,
                                             name=f"tp{i}_{j}")
                            nc.tensor.transpose(
                                tp[:], vt[:, j * 128:(j + 1) * 128], ident[:])
                            nc.vector.tensor_copy(
                                vaug[:].rearrange(
                                    "p (j a c) -> p j a c", a=2, c=65)[
                                    :, jj, :, 0:64],
                                tp[:].rearrange("p (a c) -> p a c", c=64),
                            )
                        if j0 == 2:
                            vts.pop(i)
                    return emit

                qk = [((i, 'qk'), g_granule(g, k0)) for g in range(2)
                      for k0 in range(0, 8, 2)]
                v = [((i, 'v'), g_granule(2, k0)) for k0 in range(0, 8, 2)]
                v += [((i, 'v'), t_granule(0)), ((i, 'v'), t_granule(2))]
                return qk + v

            filler = deque()

            def pop_fillers(k):
                for _ in range(k):
                    if filler:
                        filler.popleft()[1]()

            def drain_through(key):
                """Pop until no unit with the given key remains (they form a
                contiguous run in FIFO order)."""
                while any(u[0] == key for u in filler):
                    filler.popleft()[1]()

            # state deferred from the previous chunk
            pending = None  # (i, b, n, otps, pts)

            def finish_pending():
                """Tail AVs + normalization for the previous chunk; returns
                phase-4 filler units for it."""
                nonlocal pending
                if pending is None:
                    return []
                pi, pb, pn, potps, ppts = pending
                pending = None
                for j in range(max(pn - AV_DELAY, 0), pn):
                    kgj = pb * 16 + j
                    for h in range(2):
                        nc.tensor.matmul(
                            potps[h][:],
                            vaug[:, kgj * 130 + h * 65:kgj * 130 + h * 65 + 65],
                            ppts[j][:, h * TC:(h + 1) * TC],
                            start=(j == 0), stop=(j == pn - 1),
                            skip_group_check=True,
                        )
                # normalization: 1/rowsum, heads stacked into [128, TC]
                rcs = []
                for h in range(2):
                    rch = sbpool.tile([1, TC], FPR, tag=f"rc{h}", bufs=2,
                                      name=f"rc{h}_{pi}")
                    with nc.allow_low_precision(reason="softmax sums f32r"):
                        nc.vector.tensor_copy(rch[:], potps[h][64:65, :])
                    rcs.append(rch)
                ot = sbpool.tile([128, TC], BF, tag="ot", bufs=2,
                                 name=f"ot{pi}")
                for h in range(2):
                    bchh = pspool.tile([64, TC], FP, tag="m", bufs=2,
                                       name=f"bch{h}_{pi}")
                    nc.tensor.matmul(bchh[:], onesr_sb[0:1, :], rcs[h][:],
                                     start=True, stop=True)
                    bcsh = sbpool.tile([64, TC], FP, tag=f"bcs{h}", bufs=2,
                                       name=f"bcs{h}_{pi}")
                    nc.vector.reciprocal_approx_fast(out=bcsh[:], in_=bchh[:])
                    nc.vector.tensor_mul(ot[h * 64:(h + 1) * 64, :],
                                         potps[h][0:64, :], bcsh[:])

                ys = sbpool.tile([128, BT], BF, tag="ys", bufs=2,
                                 name=f"ys{pi}")

                def m_unit(m):
                    def emit():
                        yp = pspool.tile([128, TC], FP, tag="m", bufs=2,
                                         name=f"yp{pi}_{m}")
                        nc.tensor.matmul(
                            yp[:], wout_sb[:, m * 128:(m + 1) * 128],
                            ot[:], start=True, stop=True)
                        if m % 2 == 0:
                            nc.scalar.copy(
                                ys[:, m * TC:(m + 1) * TC], yp[:])
                        else:
                            nc.vector.tensor_copy(
                                ys[:, m * TC:(m + 1) * TC], yp[:])
                        if m == 7:
                            nc.sync.dma_start(
                                out=yt[pi * 128:(pi + 1) * 128, :], in_=ys[:])
                    return emit

                return [((pi, 'p4'), m_unit(m)) for m in range(8)]

            for tcx in range(NTC):
                b, qc = divmod(tcx, 4)
                t0 = tcx * TC
                n = 4 * (qc + 1)

                if tcx == 0:
                    for _, u in make_phase1_units(0):
                        u()
                    filler.extend(make_phase1_units(1))
                else:
                    # drain so phase-1(tcx) Q/K (needed by every score
                    # matmul of this chunk) is fully emitted; V granules
                    # may lag into the loop
                    drain_through((tcx, 'qk'))
                    filler.extend(finish_pending())
                    if tcx + 1 < NTC:
                        filler.extend(make_phase1_units(tcx + 1))

                otps = (
                    pspool.tile([65, TC], FP, tag="av0", bufs=1,
                                name=f"otp0_{tcx}"),
                    pspool.tile([65, TC], FP, tag="av1", bufs=1,
                                name=f"otp1_{tcx}"),
                )
                pts = {}
                for kt in range(n):
                    kg = b * 16 + kt
                    s = pspool.tile([128, 2 * TC], FP, tag="s", bufs=2,
                                    name=f"s{tcx}_{kt}")
                    for h in range(2):
                        nc.tensor.matmul(
                            s[:, h * TC:(h + 1) * TC],
                            KT[h * 64:(h + 1) * 64, kg * 128:(kg + 1) * 128],
                            QT[h * 64:(h + 1) * 64, t0:t0 + TC],
                            start=True, stop=True,
                        )
                    # V/transpose granules of this chunk must land before
                    # the diagonal AV matmuls need vaug
                    if kt >= max(n - 6, 0):
                        drain_through((tcx, 'v'))
                    # adaptive filler drain across remaining k-slots
                    slots = n - kt
                    pop_fillers(min(-(-len(filler) // slots), 3)
                                if filler else 0)
                    pt = sbpool.tile([128, 2 * TC], BF, tag="pt", bufs=6,
                                     name=f"pt{tcx}_{kt}")
                    nc.scalar.activation(pt[:], s[:], ACT.Exp, scale=0.125)
                    if kt >= n - 4:
                        # multiplicative causal mask on the bf16 exp output:
                        # cheaper on DVE and off the scores-PSUM release path
                        v = kt - (n - 4)
                        nc.vector.tensor_mul(
                            pt[:], pt[:],
                            masks_sb[:, v * 1024:(v + 1) * 1024])
                    pts[kt] = pt
                    if kt >= AV_DELAY:
                        j = kt - AV_DELAY
                        kgj = b * 16 + j
                        for h in range(2):
                            nc.tensor.matmul(
                                otps[h][:],
                                vaug[:, kgj * 130 + h * 65:
                                     kgj * 130 + h * 65 + 65],
                                pts[j][:, h * TC:(h + 1) * TC],
                                start=(j == 0), stop=(j == n - 1),
                                skip_group_check=True,
                            )
                pending = (tcx, b, n, otps, pts)

            pop_fillers(len(filler))
            for _, u in finish_pending():
                u()
    nc.compile()
    return nc


def make_in_maps(x, w_qkv, b_qkv, w_out):
    x = np.ascontiguousarray(np.asarray(x, np.float32).reshape(BT, C))
    xT = np.ascontiguousarray(x.T).astype(ml_dtypes.bfloat16)
    w_qkv = np.asarray(w_qkv, np.float32)
    b_qkv = np.asarray(b_qkv, np.float32)
    w_out = np.asarray(w_out, np.float32)

    # masks[p, v*1024 + h*512 + q] = 1 if (v*128 + p) <= q else 0
    # (multiplies the exp output: exact causal zeroing)
    kk = np.arange(128)[:, None, None, None] + 128 * np.arange(4)[None, :, None, None]
    qq = np.arange(512)[None, None, None, :]
    mask = np.where(kk <= qq, 1.0, 0.0) * np.ones((1, 1, 2, 1))
    mask = np.ascontiguousarray(
        mask.reshape(128, 4096)).astype(ml_dtypes.bfloat16)

    # wc[p, k*384 + g*128 + j] = w_qkv[k*128 + p, g*1024 + c0 + j]
    w4 = w_qkv.reshape(8, 128, 3, 1024)

    in_maps = []
    for c in range(NCORES):
        c0 = c * 128
        wcs = np.ascontiguousarray(
            w4[:, :, :, c0:c0 + 128].transpose(1, 0, 2, 3).reshape(128, 3072)
        ).astype(ml_dtypes.bfloat16)
        bq = np.ascontiguousarray(
            b_qkv.reshape(3, 1024)[:, c0:c0 + 128].T)
        in_maps.append({
            "xt": xT,
            "wc": wcs,
            "wout": np.ascontiguousarray(
                w_out[c0:c0 + 128, :]).astype(ml_dtypes.bfloat16),
            "bqkv": bq,
            "masks": mask,
            "onesr": np.ones((1, 64), np.float32),
        })
    return in_maps


_NC_CACHE = None


def kernel(x, w_qkv, b_qkv, w_out, b_out):
    global _NC_CACHE, LAST_RESULTS
    if _NC_CACHE is None:
        _NC_CACHE = build_nc()
    nc = _NC_CACHE

    in_maps = make_in_maps(x, w_qkv, b_qkv, w_out)

    res = run_bass_kernel_spmd(
        nc, in_maps, list(range(NCORES)),
        trace=bool(os.environ.get("BASS_TRACE")),
    )
    LAST_RESULTS = res

    acc = np.zeros((C, BT), np.float64)
    for out_map in res.results:
        # yt[i*128 + p, m*512 + t] -> y_partial[m*128 + p, i*512 + t]
        yp = out_map["yt"].astype(np.float32)
        yp = yp.reshape(8, 128, 8, 512).transpose(2, 1, 0, 3).reshape(C, BT)
        acc += yp
    y = acc.T.astype(np.float32) + np.asarray(b_out, np.float32)[None, :]
    return y.reshape(B, T, C)
